# revision 1
# baseline (speedup 1.0000x reference)
"""AttnBlock (GroupNorm + single-head self-attention + residual) on 8 Trainium2 cores.

Sharding: core i handles batch b = i//2 and query-half h = i%2 (2048 of 4096
pixels). Each core computes full-batch groupnorm stats + K/V^T, its half of Q,
attention over all 4096 keys for its 2048 queries, and the output projection.
Host does the final bias + residual add and gathers.

All matmuls in bf16 (fp32 PSUM accumulation); softmax statistics in fp32.
Layouts are chosen so no on-chip transpose is ever needed:
  - Q, K as [c, pixel]   (projection natural layout)
  - V^T as [pixel, c]    (computed directly: lhsT = h blocks)
  - scores S^T[k, q]     (lhsT = K blocks, rhs = Q)
  - softmax sum over k (partition dim) via ones-vector matmul
  - attention out o[c, q](lhsT = V^T blocks, rhs = E^T)
  - o-proj out[q, c']    (lhsT = o blocks, rhs = wo^T) -> per-partition 1/Z scale
"""

import numpy as np
import ml_dtypes

C = 512
HW = 4096
HWQ = 2048
CCH = 4          # channel chunks of 128
KT = 32          # key tiles of 128
QT = 4           # query tiles of 512
NCORES = 8
GS = 16          # channels per group
NGRP_CHUNK = 8   # groups per 128-channel chunk
EPS = 1e-5
SCALE = 1.0 / float(np.sqrt(C))
SCALE_H = float(SCALE ** 0.5)

_cache = {}


def _emit_body(nc, tc, bassmod, mybir, ctx, T):
    """Emit one full forward pass. T is the dict of dram tensor handles."""
    import contextlib
    bass = bassmod
    f32 = mybir.dt.float32
    bf16 = mybir.dt.bfloat16
    f8 = mybir.dt.float8e4
    f16 = mybir.dt.float16
    AF = mybir.ActivationFunctionType
    ALU = mybir.AluOpType
    DR = mybir.MatmulPerfMode.DoubleRow

    # ---------------- pools ----------------
    consts = ctx.enter_context(tc.tile_pool(name="consts", bufs=1))
    xb = ctx.enter_context(tc.tile_pool(name="xb", bufs=1))
    ps_s = ctx.enter_context(tc.tile_pool(name="ps_s", bufs=3, space="PSUM"))
    ps_o = ctx.enter_context(tc.tile_pool(name="ps_o", bufs=4, space="PSUM"))
    ps_z = ctx.enter_context(tc.tile_pool(name="ps_z", bufs=1, space="PSUM"))
    kpool = ctx.enter_context(tc.tile_pool(name="kpool", bufs=1))
    qpool = ctx.enter_context(tc.tile_pool(name="qpool", bufs=1))
    vpool = ctx.enter_context(tc.tile_pool(name="vpool", bufs=KT // 2))
    opool = ctx.enter_context(tc.tile_pool(name="opool", bufs=1))
    epool = ctx.enter_context(tc.tile_pool(name="epool", bufs=4))
    outp = ctx.enter_context(tc.tile_pool(name="outp", bufs=3))
    rzp = ctx.enter_context(tc.tile_pool(name="rzp", bufs=2))
    spool = ctx.enter_context(tc.tile_pool(name="spool", bufs=1))
    tmpp = ctx.enter_context(tc.tile_pool(name="tmpp", bufs=2))

    # ---------------- input DMAs ----------------
    cc_sb = consts.tile([128, 16], f32, tag="colconsts", name="colconsts")
    nc.sync.dma_start(out=cc_sb, in_=T["colc"][:, :])
    gadj_sb = consts.tile([128, 128], f32, tag="gadj", name="gadj")
    nc.sync.dma_start(out=gadj_sb, in_=T["gadj"][:, :])
    bv_row = consts.tile([1, C], f32, tag="bvrow", name="bvrow")
    nc.sync.dma_start(out=bv_row, in_=bass.AP(T["bvr"], 0, [[0, 1], [1, C]]))
    gnw_c = [cc_sb[:, 4 * ci + 0:4 * ci + 1] for ci in range(CCH)]
    gnb_c = [cc_sb[:, 4 * ci + 1:4 * ci + 2] for ci in range(CCH)]
    bq_c = [cc_sb[:, 4 * ci + 2:4 * ci + 3] for ci in range(CCH)]
    bk_c = [cc_sb[:, 4 * ci + 3:4 * ci + 4] for ci in range(CCH)]
    ones2_sb = consts.tile([128, 2, 16], f8, tag="ones2", name="ones2")
    nc.vector.memset(ones2_sb, 1.0)
    eps_sb = consts.tile([128, 1], f32, tag="eps", name="eps")
    nc.vector.memset(eps_sb, EPS)

    # x as fp8, one tile each: [128, ci, pixel]
    xkv_v = xb.tile([128, CCH, HW], f8, tag="xkv", name="xkv")
    nc.sync.dma_start(out=xkv_v,
                      in_=bass.AP(T["xkv"], 0, [[HW, 128], [128 * HW, CCH], [1, HW]]))
    # fp8 weights for q/k/v: [128, nm, ci, co]; bf16 weights for o
    wall = consts.tile([128, 3 * CCH * 512], f8, tag="wall", name="wall")
    nc.sync.dma_start(out=wall, in_=T["wall"][:, :])
    wv_q = {nm: wall[:, i * 2048:(i + 1) * 2048].rearrange("p (c w) -> p c w", c=CCH)
            for i, nm in enumerate(("wkt", "wvt", "wqt"))}
    wot = consts.tile([128, CCH, 512], f8, tag="wot", name="wot")
    nc.sync.dma_start(out=wot, in_=T["wotp"][:, :].rearrange("p (c w) -> p c w", c=CCH))
    xq_v = xb.tile([128, CCH, HWQ], f8, tag="xq", name="xq")
    nc.sync.dma_start(out=xq_v,
                      in_=bass.AP(T["xq"], 0, [[HWQ, 128], [128 * HWQ, CCH], [1, HWQ]]))

    # PE warm-up: HAM needs ~3.4us of activity to unthrottle, and re-throttles
    # after ~3.4us idle. Spread dummy matmuls across the stats prefix by gating
    # each round on a DVE memset that queues behind the stats work.
    wpool = ctx.enter_context(tc.tile_pool(name="wpool", bufs=2))
    ps_w = ps_z.tile([1, 512], f32, tag="z", name="z")
    _warm_state = {"first": True}

    def warm_round(last=False):
        warm = wpool.tile([128, 2, 512], f8, tag="warm", name="warm")
        nc.vector.memset(warm, 0.25)
        for i in range(7):
            nc.tensor.matmul(out=ps_w, lhsT=ones2_sb[:, :, 0:1], rhs=warm,
                             perf_mode=DR, start=_warm_state["first"],
                             stop=(last and i == 6), skip_group_check=True)
            _warm_state["first"] = False

    warm_round()

    # ---------------- groupnorm stats ----------------
    a_pc = []
    b8 = spool.tile([128, CCH], f8, tag="b8", name="b8")
    for ci in range(CCH):
        st = spool.tile([128, 2], f32, tag=f"st{ci}", name=f"st{ci}")
        nc.vector.reduce_sum(out=st[:, 0:1], in_=xkv_v[:, ci, :],
                             axis=mybir.AxisListType.X)
        scr = tmpp.tile([128, HW], f8, tag="scr", name="scr")
        nc.scalar.activation(out=scr, in_=xkv_v[:, ci, :], func=AF.Square,
                             accum_out=st[:, 1:2])
        ps_g = ps_s.tile([128, 2], f32, tag="ps", name="ps")
        nc.tensor.matmul(out=ps_g, lhsT=gadj_sb, rhs=st, start=True, stop=True)
        gs = spool.tile([128, 2], f32, tag=f"gs{ci}", name=f"gs{ci}")
        nc.scalar.mul(out=gs, in_=ps_g, mul=1.0 / (GS * HW))   # [mu | E[x^2]]
        var = spool.tile([128, 1], f32, tag=f"var{ci}", name=f"var{ci}")
        nc.vector.tensor_mul(var, gs[:, 0:1], gs[:, 0:1])
        nc.vector.tensor_sub(var, gs[:, 1:2], var)
        sd = spool.tile([128, 1], f32, tag=f"sd{ci}", name=f"sd{ci}")
        nc.scalar.activation(out=sd, in_=var, func=AF.Sqrt, bias=eps_sb, scale=1.0)
        rstd = spool.tile([128, 1], f32, tag=f"rstd{ci}", name=f"rstd{ci}")
        nc.vector.reciprocal(out=rstd, in_=sd)
        a = spool.tile([128, 1], f32, tag=f"apc{ci}", name=f"apc{ci}")
        nc.vector.tensor_mul(a, rstd, gnw_c[ci])
        b = spool.tile([128, 1], f32, tag=f"bpc{ci}", name=f"bpc{ci}")
        nc.vector.tensor_mul(b, gs[:, 0:1], a)
        nc.vector.tensor_sub(b, gnb_c[ci], b)
        nc.vector.tensor_copy(b8[:, ci:ci + 1], b)
        a_pc.append(a)
        warm_round(last=(ci == CCH - 1))

    # effective projection biases: beta = W @ b + bias  (original weights, fp8 matvecs)
    bqeff, bkeff = [], []
    for co in range(CCH):
        psq = ps_s.tile([128, 1], f32, tag="ps", name="ps")
        psk = ps_s.tile([128, 1], f32, tag="ps", name="ps")
        for ci in range(CCH):
            nc.tensor.matmul(out=psk, lhsT=wv_q["wkt"][:, ci, co * 128:(co + 1) * 128],
                             rhs=b8[:, ci:ci + 1], start=(ci == 0), stop=(ci == CCH - 1))
        for ci in range(CCH):
            nc.tensor.matmul(out=psq, lhsT=wv_q["wqt"][:, ci, co * 128:(co + 1) * 128],
                             rhs=b8[:, ci:ci + 1], start=(ci == 0), stop=(ci == CCH - 1))
        bk = spool.tile([128, 1], f32, tag=f"bke{co}", name=f"bke{co}")
        nc.scalar.activation(out=bk, in_=psk, func=AF.Identity, bias=bk_c[co], scale=SCALE_H)
        bq = spool.tile([128, 1], f32, tag=f"bqe{co}", name=f"bqe{co}")
        nc.scalar.activation(out=bq, in_=psq, func=AF.Identity, bias=bq_c[co], scale=SCALE_H)
        bqeff.append(bq)
        bkeff.append(bk)
    psv = ps_s.tile([1, C], f32, tag="ps", name="ps")
    for ci in range(CCH):
        nc.tensor.matmul(out=psv, lhsT=b8[:, ci:ci + 1], rhs=wv_q["wvt"][:, ci, :],
                         start=(ci == 0), stop=(ci == CCH - 1))
    bve_row = spool.tile([1, C], f32, tag="bverow", name="bverow")
    nc.vector.tensor_add(bve_row, psv, bv_row)
    nc.sync.dma_start(out=bass.AP(T["vbb"], 0, [[1, C]]), in_=bve_row)
    bvb_sb = consts.tile([128, C], f32, tag="bvb", name="bvb")
    nc.sync.dma_start(out=bvb_sb, in_=bass.AP(T["vbb"], 0, [[0, 128], [1, C]]))

    # scale q/k/v weight rows by a (in place, after the beta matvecs read them);
    # alternate DVE/ACT so the chain before the first projection halves
    for nm in ("wkt", "wvt", "wqt"):
        for ci in range(CCH):
            if ci % 2 == 0:
                nc.vector.tensor_scalar_mul(wv_q[nm][:, ci, :], wv_q[nm][:, ci, :], a_pc[ci])
            else:
                nc.scalar.activation(out=wv_q[nm][:, ci, :], in_=wv_q[nm][:, ci, :],
                                     func=AF.Copy, scale=a_pc[ci])

    # ---------------- projections (fp8 DoubleRow) ----------------
    # K: [128, 4(co), 4096] fp8, scaled by C**-0.25
    ksb = kpool.tile([128, CCH, HW], f8, tag="ksb", name="ksb")
    for co in range(CCH):
        for pt in range(HW // 512):
            ps = ps_s.tile([128, 512], f32, tag="ps", name="ps")
            for j in range(2):
                nc.tensor.matmul(out=ps,
                                 lhsT=wv_q["wkt"][:, 2 * j:2 * j + 2, co * 128:(co + 1) * 128],
                                 rhs=xkv_v[:, 2 * j:2 * j + 2, pt * 512:(pt + 1) * 512],
                                 perf_mode=DR, start=(j == 0), stop=(j == 1))
            nc.vector.tensor_scalar(out=ksb[:, co, pt * 512:(pt + 1) * 512], in0=ps,
                                    scalar1=SCALE_H, scalar2=bkeff[co],
                                    op0=ALU.mult, op1=ALU.add)
    # V^T: 16 pair tiles [128, 2, 512] fp8
    vsb = [vpool.tile([128, 2, C], f8, tag="vt", name="vt") for _ in range(KT // 2)]
    for kt in range(KT):
        ps = ps_s.tile([128, 512], f32, tag="ps", name="ps")
        for j in range(2):
            nc.tensor.matmul(out=ps,
                             lhsT=xkv_v[:, 2 * j:2 * j + 2, kt * 128:(kt + 1) * 128],
                             rhs=wv_q["wvt"][:, 2 * j:2 * j + 2, :],
                             perf_mode=DR, start=(j == 0), stop=(j == 1))
        nc.vector.tensor_add(vsb[kt // 2][:, kt % 2, :], ps, bvb_sb)
    # Q: [128, 4(co), 2048] fp8, scaled by C**-0.25
    qsb = qpool.tile([128, CCH, HWQ], f8, tag="qsb", name="qsb")
    for co in range(CCH):
        for pt in range(HWQ // 512):
            ps = ps_s.tile([128, 512], f32, tag="ps", name="ps")
            for j in range(2):
                nc.tensor.matmul(out=ps,
                                 lhsT=wv_q["wqt"][:, 2 * j:2 * j + 2, co * 128:(co + 1) * 128],
                                 rhs=xq_v[:, 2 * j:2 * j + 2, pt * 512:(pt + 1) * 512],
                                 perf_mode=DR, start=(j == 0), stop=(j == 1))
            nc.vector.tensor_scalar(out=qsb[:, co, pt * 512:(pt + 1) * 512], in0=ps,
                                    scalar1=SCALE_H, scalar2=bqeff[co],
                                    op0=ALU.mult, op1=ALU.add)

    # ---------------- attention (+ deferred per-tile output projection) ----------------
    OSC = 1.0 / 32.0   # o scaled into fp8 range; undone via the 1/Z multiply

    def emit_oproj(qt, o_qt):
        for qc in range(4):
            ps = ps_s.tile([128, 512], f32, tag="ps", name="ps")
            for j in range(2):
                nc.tensor.matmul(out=ps, lhsT=o_qt[:, 2 * j:2 * j + 2, qc * 128:(qc + 1) * 128],
                                 rhs=wot[:, 2 * j:2 * j + 2, :], perf_mode=DR,
                                 start=(j == 0), stop=(j == 1))
            rzc = rzp.tile([128, 1], f32, tag="rzc", name="rzc")
            row0z = qt * 512 + qc * 128
            nc.sync.dma_start(out=rzc, in_=T["zb"][row0z:row0z + 128, :])
            ot = outp.tile([128, 512], f16, tag="ot", name="ot")
            nc.vector.tensor_scalar(out=ot, in0=ps, scalar1=rzc, scalar2=1.0 / OSC,
                                    op0=ALU.mult, op1=ALU.mult)
            nc.sync.dma_start(out=T["outt"][row0z:row0z + 128, :], in_=ot)

    pending = None
    for qt in range(QT):
        ps_ot = [ps_o.tile([128, 512], f32, tag="pso", name="pso") for _ in range(CCH)]
        ps_zt = ps_z.tile([1, 512], f32, tag="z", name="z")
        prev_pair = None
        e_pair = None
        for kt in range(KT):
            pair, r = kt // 2, kt % 2
            ps_st = ps_s.tile([128, 512], f32, tag="ps", name="ps")
            for j in range(2):
                nc.tensor.matmul(out=ps_st,
                                 lhsT=ksb[:, 2 * j:2 * j + 2, kt * 128:(kt + 1) * 128],
                                 rhs=qsb[:, 2 * j:2 * j + 2, qt * 512:(qt + 1) * 512],
                                 perf_mode=DR, start=(j == 0), stop=(j == 1))
            if r == 0:
                e_pair = epool.tile([128, 2, 512], f8, tag="e", name="e")
            nc.scalar.activation(out=e_pair[:, r, :], in_=ps_st, func=AF.Exp)
            if r == 0 and prev_pair is not None:
                ppair, pe = prev_pair
                nc.tensor.matmul(out=ps_zt, lhsT=ones2_sb[:, :, 0:1], rhs=pe, perf_mode=DR,
                                 start=(ppair == 0), stop=False, skip_group_check=True)
                for cc in range(CCH):
                    nc.tensor.matmul(out=ps_ot[cc],
                                     lhsT=vsb[ppair][:, :, cc * 128:(cc + 1) * 128],
                                     rhs=pe, perf_mode=DR, start=(ppair == 0),
                                     stop=False, skip_group_check=True)
            if r == 1:
                prev_pair = (pair, e_pair)
            if kt == 5 and pending is not None:
                emit_oproj(*pending)
                pending = None
        ppair, pe = prev_pair
        nc.tensor.matmul(out=ps_zt, lhsT=ones2_sb[:, :, 0:1], rhs=pe, perf_mode=DR,
                         start=False, stop=True, skip_group_check=True)
        for cc in range(CCH):
            nc.tensor.matmul(out=ps_ot[cc],
                             lhsT=vsb[ppair][:, :, cc * 128:(cc + 1) * 128],
                             rhs=pe, perf_mode=DR, start=False, stop=True,
                             skip_group_check=True)
        rz_row = rzp.tile([1, 512], f32, tag="rzrow", name="rzrow")
        nc.vector.reciprocal(out=rz_row, in_=ps_zt)
        nc.sync.dma_start(out=T["zb"][qt * 512:(qt + 1) * 512, :], in_=rz_row)
        o_qt = opool.tile([128, CCH, 512], f8, tag=f"o{qt}", name=f"o{qt}")
        if qt < QT - 1:
            for cc in range(CCH):
                nc.vector.tensor_scalar_mul(o_qt[:, cc, :], ps_ot[cc], OSC)
        pending = (qt, o_qt)
    # final tile: per-qc slice copies interleaved with its output projection
    qt, o_qt = pending
    for qc in range(4):
        for cc in range(CCH):
            nc.vector.tensor_scalar_mul(o_qt[:, cc, qc * 128:(qc + 1) * 128],
                                        ps_ot[cc][:, qc * 128:(qc + 1) * 128], OSC)
        ps = ps_s.tile([128, 512], f32, tag="ps", name="ps")
        for j in range(2):
            nc.tensor.matmul(out=ps, lhsT=o_qt[:, 2 * j:2 * j + 2, qc * 128:(qc + 1) * 128],
                             rhs=wot[:, 2 * j:2 * j + 2, :], perf_mode=DR,
                             start=(j == 0), stop=(j == 1))
        rzc = rzp.tile([128, 1], f32, tag="rzc", name="rzc")
        row0z = qt * 512 + qc * 128
        nc.sync.dma_start(out=rzc, in_=T["zb"][row0z:row0z + 128, :])
        ot = outp.tile([128, 512], f16, tag="ot", name="ot")
        nc.vector.tensor_scalar(out=ot, in0=ps, scalar1=rzc, scalar2=1.0 / OSC,
                                op0=ALU.mult, op1=ALU.mult)
        nc.sync.dma_start(out=T["outt"][row0z:row0z + 128, :], in_=ot)


def build_program(repeat=1):
    import concourse.bacc as bacc
    import concourse.tile as tile
    import concourse.bass as bass
    from concourse import mybir
    import contextlib

    f32 = mybir.dt.float32
    bf16 = mybir.dt.bfloat16
    nc = bacc.Bacc(None, target_bir_lowering=False)
    T = {}
    f8 = mybir.dt.float8e4
    T["xkv"] = nc.dram_tensor("xkv", [C, HW], f8, kind="ExternalInput")
    T["xq"] = nc.dram_tensor("xq", [C, HWQ], f8, kind="ExternalInput")
    T["wall"] = nc.dram_tensor("wall", [128, 12 * 512], f8, kind="ExternalInput")
    T["wotp"] = nc.dram_tensor("wotp", [128, 4 * 512], f8, kind="ExternalInput")
    T["colc"] = nc.dram_tensor("colc", [128, 16], f32, kind="ExternalInput")
    T["bvr"] = nc.dram_tensor("bvr", [C], f32, kind="ExternalInput")
    T["gadj"] = nc.dram_tensor("gadj", [128, 128], f32, kind="ExternalInput")
    T["outt"] = nc.dram_tensor("outt", [HWQ, C], mybir.dt.float16, kind="ExternalOutput")
    T["zb"] = nc.dram_tensor("zb", [HWQ, 1], f32)
    T["vbb"] = nc.dram_tensor("vbb", [C], f32)

    with tile.TileContext(nc) as tc:
        for _ in range(repeat):
            with contextlib.ExitStack() as ctx:
                _emit_body(nc, tc, bass, mybir, ctx, T)
    nc.finalize()
    return nc


def make_in_maps(inputs):
    """Host-side sharding: per-core input dicts."""
    bf = ml_dtypes.bfloat16
    x = np.ascontiguousarray(np.asarray(inputs["x"], dtype=np.float32))
    B = x.shape[0]
    xf = x.reshape(B, C, HW)
    f8 = ml_dtypes.float8_e4m3
    wT8 = {nm: np.asarray(inputs[nm], np.float32).T.astype(f8)
           for nm in ("wq", "wk", "wv")}
    wall = np.empty((128, 12 * 512), f8)
    for i, nm in enumerate(("wk", "wv", "wq")):
        for ci in range(CCH):
            wall[:, i * 2048 + ci * 512:i * 2048 + (ci + 1) * 512] = \
                wT8[nm][ci * 128:(ci + 1) * 128, :]
    woT = np.asarray(inputs["wo"], np.float32).T.astype(f8)
    wotp = np.empty((128, 4 * 512), f8)
    for ci in range(CCH):
        wotp[:, ci * 512:(ci + 1) * 512] = woT[ci * 128:(ci + 1) * 128, :]
    colc = np.empty((128, 16), np.float32)
    for ci in range(CCH):
        sl = slice(ci * 128, (ci + 1) * 128)
        colc[:, 4 * ci + 0] = np.asarray(inputs["gn_w"], np.float32)[sl]
        colc[:, 4 * ci + 1] = np.asarray(inputs["gn_b"], np.float32)[sl]
        colc[:, 4 * ci + 2] = np.asarray(inputs["bq"], np.float32)[sl] * SCALE_H
        colc[:, 4 * ci + 3] = np.asarray(inputs["bk"], np.float32)[sl] * SCALE_H
    com = {
        "wall": np.ascontiguousarray(wall),
        "wotp": np.ascontiguousarray(wotp),
        "colc": np.ascontiguousarray(colc),
        "bvr": np.ascontiguousarray(np.asarray(inputs["bv"], np.float32)),
        "gadj": np.ascontiguousarray(
            (np.arange(128)[:, None] // GS == np.arange(128)[None, :] // GS).astype(np.float32)),
    }
    in_maps = []
    for core in range(NCORES):
        b, half = core // 2, core % 2
        m = dict(com)
        m["xkv"] = np.ascontiguousarray(xf[b]).astype(f8)
        m["xq"] = np.ascontiguousarray(xf[b][:, half * HWQ:(half + 1) * HWQ]).astype(f8)
        in_maps.append(m)
    return in_maps


def assemble(inputs, results):
    x = np.asarray(inputs["x"], dtype=np.float32)
    B = x.shape[0]
    xf = x.reshape(B, C, HW)
    bo = np.asarray(inputs["bo"], np.float32)
    out = np.empty((B, C, HW), np.float32)
    for core in range(NCORES):
        b, half = core // 2, core % 2
        out[b][:, half * HWQ:(half + 1) * HWQ] = results[core]["outt"].T.astype(np.float32)
    out += bo[None, :, None]
    out += xf
    return out.reshape(x.shape)


def kernel(**inputs):
    from concourse.bass_utils import run_bass_kernel_spmd
    if "nc" not in _cache:
        _cache["nc"] = build_program(repeat=1)
    nc = _cache["nc"]
    in_maps = make_in_maps(inputs)
    res = run_bass_kernel_spmd(nc, in_maps, list(range(NCORES)))
    return assemble(inputs, res.results)

